# revision 2
# baseline (speedup 1.0000x reference)
"""GCN message-passing kernel (nn_Encoder_953482739902) for 8 TRN2 NeuronCores.

Computation (PyG GCNConv + mish):
    deg  = in-degree(col) + 1 (self-loops)
    dinv = deg^-1/2
    agg[t] = sum_{e: col(e)=t} dinv[row_e] * x[row_e] + dinv[t] * x[t]
    out  = mish(dinv[t] * (agg @ W) + b)

Distribution: targets (and output rows) sharded 8 ways; each core owns 12500
targets and the edges pointing at them (edge partition by target node). Every
core receives the full packed feature table.

Per-core device pipeline:
  - sources gathered via dma_gather (int16 indices; the 100k-row table is
    split in 4 chunks of 32768 rows to fit int16 indexing). deg(src) rides
    along mantissa-packed into the low 8 bits of feature 0, extracted on
    device (int ops + Ln/Exp for deg^-1/2).
  - segment-sum over targets via valued one-hot matmuls (O[e, t] =
    (iota==tloc)*dinv_src) accumulating in a PSUM bank per 512-target window.
  - finalize: agg @ W, scale by dinv_tgt, exact mish via
    z * a / (a + 2) with a = u^2 + 2u, u = exp(z).

Host side does index work only: bincount (degree), sorting/partitioning,
int casts, layout packing. All floating-point math runs on device.
"""

import numpy as np

N_NODES = 100000
IN_CH = 128
N_CORES = 8
TPC = 12500              # targets per core
TPAD = 12800             # padded (25 windows x 512)
NW = TPAD // 512         # 25 PSUM windows per core
NCHUNK = 4
CHUNK = 32768
NROWS_PAD = NCHUNK * CHUNK
GU_MAX = 16              # subtiles (128 idx each) per dma_gather


def _build_schedule(row, col):
    """Partition/sort edges; return per-core host arrays + static sizes.

    row/col: int64 [E_tot] including self-loops.
    Static shapes (shared across cores): per (window, chunk) slot subtile
    count = max over cores, rounded up.
    """
    E_tot = row.shape[0]
    core = col // TPC
    tgt_local = (col - core * TPC).astype(np.int64)
    window = tgt_local >> 9
    chunk = row >> 15

    # counts per (core, window, chunk)
    key = ((core * NW + window) * NCHUNK + chunk).astype(np.int64)
    counts = np.bincount(key, minlength=N_CORES * NW * NCHUNK).reshape(
        N_CORES, NW, NCHUNK
    )
    s_slot = np.maximum(1, -(-counts.max(axis=0) // 128))  # [NW, NCHUNK] subtiles

    # per-slot gather units of <= GU_MAX subtiles
    gu_list = []  # (w, c, subtile_base_global, n_subtiles)
    sub_base = np.zeros((NW, NCHUNK), np.int64)
    nsub_tot = 0
    for w in range(NW):
        for c in range(NCHUNK):
            sub_base[w, c] = nsub_tot
            nsub_tot += int(s_slot[w, c])
    for w in range(NW):
        for c in range(NCHUNK):
            s = int(s_slot[w, c])
            off = 0
            while off < s:
                su = min(GU_MAX, s - off)
                gu_list.append((w, c, int(sub_base[w, c]) + off, su))
                off += su

    # order of edges inside the concatenated per-core stream
    order = np.lexsort((tgt_local, chunk, window, core))
    ro, tl, wo, co, ko = (
        row[order], tgt_local[order], window[order], core[order], chunk[order]
    )

    idx_cols = nsub_tot * 8  # int16 cols: 128 idx/subtile / 16
    idx16 = np.zeros((N_CORES, 128, idx_cols), np.int16)
    tloc = np.full((N_CORES, 128, nsub_tot), -1.0, np.float32)

    # slot start offsets within the sorted stream, per core
    cum = np.zeros((N_CORES, NW, NCHUNK + 1), np.int64)
    cum[:, :, 1:] = np.cumsum(counts, axis=2)
    wcum = np.zeros((N_CORES, NW + 1), np.int64)
    wcum[:, 1:] = np.cumsum(counts.sum(axis=2), axis=1)
    ccum = np.zeros(N_CORES + 1, np.int64)
    ccum[1:] = np.cumsum(counts.sum(axis=(1, 2)))

    for cc in range(N_CORES):
        for w in range(NW):
            for ch in range(NCHUNK):
                a = ccum[cc] + wcum[cc, w] + cum[cc, w, ch]
                b = ccum[cc] + wcum[cc, w] + cum[cc, w, ch + 1]
                n = b - a
                if n == 0:
                    continue
                srcs = (ro[a:b] - ch * CHUNK).astype(np.int16)
                tls = (tl[a:b] - (w << 9)).astype(np.float32)
                sb = int(sub_base[w, ch])
                npad = int(s_slot[w, ch]) * 128
                sv = np.zeros(npad, np.int16)
                tv = np.full(npad, -1.0, np.float32)
                sv[:n] = srcs
                tv[:n] = tls
                # idx: logical i -> (i%16, i//16), replicated 8x over partitions
                wrapped = sv.reshape(-1, 16).T  # [16, npad/16]
                idx16[cc, :, sb * 8 : sb * 8 + npad // 16] = np.tile(wrapped, (8, 1))
                # tloc: logical i -> (partition i%128, subtile i//128)
                tloc[cc, :, sb : sb + npad // 128] = tv.reshape(-1, 128).T

    return idx16, tloc, gu_list, nsub_tot, int(s_slot.sum())


def _fix_act_table_loads(nc):
    """All activations used here (Ln, Exp, Copy-family) live in the single
    'natural_log_exp_and_others' table set; the default per-function chooser
    alternates between sets, inserting a ~100us table reload before nearly
    every activation. Retarget every load to the combined set and drop the
    now-redundant repeats (keep the first load per basic block)."""
    import concourse.mybir as mybir
    from concourse.hw_specs import get_activation_tables

    tables = get_activation_tables(nc.m.arch)
    names = list(tables.keys())
    target = "natural_log_exp_and_others"
    target_id = names.index(target)
    allowed = tables[target]
    for f in nc.m.functions:
        for blk in f.blocks:
            insts = blk.instructions
            for inst in insts:
                if isinstance(inst, mybir.InstActivation):
                    assert inst.func in allowed, inst.func
            kept = []
            seen_load = False
            for inst in insts:
                if isinstance(inst, mybir.InstLoadActFuncSet):
                    si = inst.sync_info
                    has_sync = si is not None and (si.on_wait or si.on_update)
                    if seen_load and not has_sync:
                        continue
                    inst.act_func_set_id = target_id
                    seen_load = True
                kept.append(inst)
            if len(kept) != len(insts):
                insts[:] = kept


def _build_bass(nsub_tot, gu_list, has_bias):
    import concourse.bacc as bacc
    import concourse.mybir as mybir
    from concourse.tile import TileContext

    AF = mybir.ActivationFunctionType
    OP = mybir.AluOpType

    nc = bacc.Bacc("TRN2", target_bir_lowering=False, debug=False,
                   num_devices=N_CORES)
    _orig_compile = nc.compile

    def _compile_with_fix():
        _orig_compile()
        _fix_act_table_loads(nc)

    nc.compile = _compile_with_fix
    xpk_d = nc.dram_tensor("xpk", [NROWS_PAD, IN_CH], mybir.dt.float32,
                           kind="ExternalInput")
    idx_d = nc.dram_tensor("idx16", [128, nsub_tot * 8], mybir.dt.int16,
                           kind="ExternalInput")
    tloc_d = nc.dram_tensor("tloc", [128, nsub_tot], mybir.dt.float32,
                            kind="ExternalInput")
    iota_d = nc.dram_tensor("iota512", [128, 512], mybir.dt.float32,
                            kind="ExternalInput")
    w_d = nc.dram_tensor("Wm", [IN_CH, IN_CH], mybir.dt.float32,
                         kind="ExternalInput")
    degt_d = nc.dram_tensor("degt", [128, TPAD // 128], mybir.dt.float32,
                            kind="ExternalInput")
    if has_bias:
        b_d = nc.dram_tensor("bb", [128, 512], mybir.dt.float32,
                             kind="ExternalInput")
    out_d = nc.dram_tensor("out", [TPAD, IN_CH], mybir.dt.float32,
                           kind="ExternalOutput")

    with TileContext(nc) as tc:
        with (
            tc.tile_pool(name="const", bufs=1) as cp,
            tc.tile_pool(name="gbuf", bufs=3) as gp,
            tc.tile_pool(name="obuf", bufs=4) as op_,
            tc.tile_pool(name="small", bufs=3) as sp,
            tc.tile_pool(name="fin", bufs=2) as fp,
            tc.tile_pool(name="psw", bufs=2, space="PSUM") as pwp,
            tc.tile_pool(name="ps2", bufs=2, space="PSUM") as p2p,
        ):
            iota = cp.tile([128, 512], mybir.dt.float32)
            nc.sync.dma_start(out=iota[:], in_=iota_d[:])
            wsb = cp.tile([IN_CH, IN_CH], mybir.dt.float32)
            nc.sync.dma_start(out=wsb[:], in_=w_d[:])
            idxt = cp.tile([128, nsub_tot * 8], mybir.dt.int16)
            nc.sync.dma_start(out=idxt[:], in_=idx_d[:])
            tlct = cp.tile([128, nsub_tot], mybir.dt.float32)
            nc.sync.dma_start(out=tlct[:], in_=tloc_d[:])
            degt = cp.tile([128, TPAD // 128], mybir.dt.float32)
            nc.sync.dma_start(out=degt[:], in_=degt_d[:])
            if has_bias:
                bsb = cp.tile([128, 512], mybir.dt.float32)
                nc.sync.dma_start(out=bsb[:], in_=b_d[:])
            lnt = cp.tile([128, TPAD // 128], mybir.dt.float32)
            nc.scalar.activation(lnt[:], degt[:], AF.Ln)
            dinvt = cp.tile([128, TPAD // 128], mybir.dt.float32)
            nc.scalar.activation(dinvt[:], lnt[:], AF.Exp, scale=-0.5)

            # group gather units by window
            gus_by_w = [[] for _ in range(NW)]
            for (w, c, sb, su) in gu_list:
                gus_by_w[w].append((c, sb, su))

            for w in range(NW):
                psw = pwp.tile([128, 512], mybir.dt.float32, tag="psw",
                               space="PSUM")
                n_mm = sum(su for (_, _, su) in gus_by_w[w])
                mm = 0
                for (c, sb, su) in gus_by_w[w]:
                    g = gp.tile([128, GU_MAX, 128], mybir.dt.float32, tag="g")
                    nc.gpsimd.dma_gather(
                        out_ap=g[:, :su, :],
                        in_ap=xpk_d[c * CHUNK : (c + 1) * CHUNK, :],
                        idxs_ap=idxt[:, sb * 8 : (sb + su) * 8],
                        num_idxs=su * 128,
                        num_idxs_reg=su * 128,
                        elem_size=IN_CH,
                        single_packet=False,
                    )
                    degs_i = sp.tile([128, GU_MAX], mybir.dt.int32, tag="di")
                    nc.vector.tensor_scalar(
                        out=degs_i[:, :su],
                        in0=g[:, :su, 0].bitcast(mybir.dt.int32),
                        scalar1=0xFF, scalar2=None, op0=OP.bitwise_and)
                    degs_f = sp.tile([128, GU_MAX], mybir.dt.float32, tag="df")
                    nc.vector.tensor_copy(out=degs_f[:, :su], in_=degs_i[:, :su])
                    lns = sp.tile([128, GU_MAX], mybir.dt.float32, tag="ln")
                    nc.scalar.activation(lns[:, :su], degs_f[:, :su], AF.Ln)
                    dinvs = sp.tile([128, GU_MAX], mybir.dt.float32, tag="dv")
                    nc.scalar.activation(dinvs[:, :su], lns[:, :su], AF.Exp,
                                         scale=-0.5)
                    for j in range(su):
                        O = op_.tile([128, 512], mybir.dt.float32, tag="O")
                        nc.vector.tensor_scalar(
                            out=O[:], in0=iota[:],
                            scalar1=tlct[:, sb + j : sb + j + 1],
                            scalar2=dinvs[:, j : j + 1],
                            op0=OP.is_equal, op1=OP.mult)
                        nc.tensor.matmul(out=psw[:], lhsT=g[:, j, :], rhs=O[:],
                                         start=(mm == 0), stop=(mm == n_mm - 1))
                        mm += 1

                accs = fp.tile([128, 512], mybir.dt.float32, tag="acc")
                nc.vector.tensor_copy(out=accs[:], in_=psw[:])
                ps2 = p2p.tile([128, 512], mybir.dt.float32, tag="ps2",
                               space="PSUM")
                zt = fp.tile([128, 512], mybir.dt.float32, tag="zt")
                for j4 in range(4):
                    sl = slice(j4 * 128, (j4 + 1) * 128)
                    nc.tensor.matmul(out=ps2[:, sl], lhsT=accs[:, sl],
                                     rhs=wsb[:], start=True, stop=True)
                    nc.vector.tensor_scalar(
                        out=zt[:, sl], in0=ps2[:, sl],
                        scalar1=dinvt[:, 4 * w + j4 : 4 * w + j4 + 1],
                        scalar2=None, op0=OP.mult)
                if has_bias:
                    nc.vector.tensor_tensor(out=zt[:], in0=zt[:], in1=bsb[:],
                                            op=OP.add)
                u = fp.tile([128, 512], mybir.dt.float32, tag="u")
                nc.scalar.activation(u[:], zt[:], AF.Exp)
                a1 = fp.tile([128, 512], mybir.dt.float32, tag="a1")
                nc.vector.tensor_scalar(out=a1[:], in0=u[:], scalar1=2.0,
                                        scalar2=None, op0=OP.add)
                a = fp.tile([128, 512], mybir.dt.float32, tag="a")
                nc.vector.tensor_tensor(out=a[:], in0=a1[:], in1=u[:],
                                        op=OP.mult)
                den = fp.tile([128, 512], mybir.dt.float32, tag="den")
                nc.vector.tensor_scalar(out=den[:], in0=a[:], scalar1=2.0,
                                        scalar2=None, op0=OP.add)
                rden = fp.tile([128, 512], mybir.dt.float32, tag="rden")
                nc.vector.reciprocal_approx_fast(out=rden[:], in_=den[:])
                m = fp.tile([128, 512], mybir.dt.float32, tag="m")
                nc.vector.tensor_tensor(out=m[:], in0=a[:], in1=rden[:],
                                        op=OP.mult)
                mz = fp.tile([128, 512], mybir.dt.float32, tag="mz")
                nc.vector.tensor_tensor(out=mz[:], in0=m[:], in1=zt[:],
                                        op=OP.mult)
                for j4 in range(4):
                    nc.sync.dma_start(
                        out=out_d[w * 512 + j4 * 128 : w * 512 + (j4 + 1) * 128, :],
                        in_=mz[:, j4 * 128 : (j4 + 1) * 128])
    nc.finalize()
    return nc


class _Runner:
    """PJRT runner (axon): jit once, device-resident inputs, reusable."""

    def __init__(self, nc):
        import jax
        import concourse.mybir as mybir
        from jax.sharding import Mesh, PartitionSpec
        from jax.experimental.shard_map import shard_map
        from concourse import bass2jax
        from concourse.bass2jax import _bass_exec_p, install_neuronx_cc_hook

        install_neuronx_cc_hook()
        self.nc = nc
        partition_name = (
            nc.partition_id_tensor.name if nc.partition_id_tensor else None
        )
        in_names, out_names, out_avals, zero_outs = [], [], [], []
        for alloc in nc.m.functions[0].allocations:
            if not isinstance(alloc, mybir.MemoryLocationSet):
                continue
            name = alloc.memorylocations[0].name
            if alloc.kind == "ExternalInput":
                if name != partition_name:
                    in_names.append(name)
            elif alloc.kind == "ExternalOutput":
                shape = tuple(alloc.tensor_shape)
                dtype = mybir.dt.np(alloc.dtype)
                out_names.append(name)
                out_avals.append(jax.core.ShapedArray(shape, dtype))
                zero_outs.append(np.zeros(shape, dtype))
        self.in_names, self.out_names = in_names, out_names
        all_in = list(in_names) + list(out_names)
        if partition_name is not None:
            all_in.append(partition_name)

        def _body(*args):
            operands = list(args)
            if partition_name is not None:
                operands.append(bass2jax.partition_id_tensor())
            return tuple(_bass_exec_p.bind(
                *operands,
                out_avals=tuple(out_avals),
                in_names=tuple(all_in),
                out_names=tuple(out_names),
                lowering_input_output_aliases=(),
                sim_require_finite=True,
                sim_require_nnan=True,
                nc=nc,
            ))

        devices = jax.devices()[:N_CORES]
        mesh = Mesh(np.asarray(devices), ("core",))
        n_in = len(in_names) + len(out_names)
        self.fn = jax.jit(
            shard_map(_body, mesh=mesh,
                      in_specs=(PartitionSpec("core"),) * n_in,
                      out_specs=(PartitionSpec("core"),) * len(out_names),
                      check_rep=False),
            keep_unused=True)
        self.zero_outs = zero_outs
        self.jax = jax

    def stage(self, in_maps):
        args = []
        for name in self.in_names:
            args.append(np.concatenate(
                [np.asarray(m[name]) for m in in_maps], axis=0))
        for z in self.zero_outs:
            args.append(np.concatenate([z] * N_CORES, axis=0))
        self._dev_args = [self.jax.device_put(a) for a in args]
        for a in self._dev_args:
            a.block_until_ready()

    def run(self):
        outs = self.fn(*self._dev_args)
        for o in outs:
            o.block_until_ready()
        return outs

    def results(self, outs):
        per_core = [dict() for _ in range(N_CORES)]
        for i, name in enumerate(self.out_names):
            arr = np.asarray(outs[i])
            for c, piece in enumerate(np.split(arr, N_CORES, axis=0)):
                per_core[c][name] = piece
        return per_core


_CACHE = {}


def _prepare(x, edge_index, W, b):
    x = np.asarray(x, dtype=np.float32)
    edge_index = np.asarray(edge_index)
    W = np.asarray(W, dtype=np.float32)
    b = np.asarray(b, dtype=np.float32)
    N = x.shape[0]
    assert N == N_NODES and x.shape[1] == IN_CH

    row = edge_index[0].astype(np.int64)
    col = edge_index[1].astype(np.int64)
    loops = np.arange(N, dtype=np.int64)
    row_all = np.concatenate([row, loops])
    col_all = np.concatenate([col, loops])

    deg = np.bincount(col_all, minlength=N).astype(np.int64)
    assert deg.max() <= 255, "degree exceeds 8-bit mantissa packing"

    # packed feature table: deg in low 8 mantissa bits of feature 0
    xpk = np.zeros((NROWS_PAD, IN_CH), np.float32)
    xpk[:N] = x
    bits = xpk[:, 0].view(np.int32)
    bits[:N] = (bits[:N] & ~np.int32(0xFF)) | deg.astype(np.int32)
    bits[N:] = 1  # harmless deg=1 for padding rows

    idx16, tloc, gu_list, nsub_tot, _ = _build_schedule(row_all, col_all)

    degt = np.ones((N_CORES, 128, TPAD // 128), np.float32)
    for c in range(N_CORES):
        d = deg[c * TPC : (c + 1) * TPC].astype(np.float32)
        dp = np.ones(TPAD, np.float32)
        dp[:TPC] = d
        degt[c] = dp.reshape(-1, 128).T

    iota512 = np.broadcast_to(
        np.arange(512, dtype=np.float32), (128, 512)).copy()
    has_bias = bool(np.any(b != 0))

    in_maps = []
    for c in range(N_CORES):
        m = {
            "xpk": xpk,
            "idx16": np.ascontiguousarray(idx16[c]),
            "tloc": np.ascontiguousarray(tloc[c]),
            "iota512": iota512,
            "Wm": W,
            "degt": np.ascontiguousarray(degt[c]),
        }
        if has_bias:
            m["bb"] = np.tile(b[None, :], (128, 4)).astype(np.float32)
        in_maps.append(m)

    key = (nsub_tot, tuple(gu_list), has_bias)
    if key not in _CACHE:
        nc = _build_bass(nsub_tot, gu_list, has_bias)
        runner = _Runner(nc)
        _CACHE.clear()
        _CACHE[key] = runner
    return _CACHE[key], in_maps


def kernel(x, edge_index, W, b):
    runner, in_maps = _prepare(x, edge_index, W, b)
    runner.stage(in_maps)
    outs = runner.run()
    res = runner.results(outs)
    return np.concatenate(
        [res[c]["out"][:TPC] for c in range(N_CORES)], axis=0)


# revision 3
# speedup vs baseline: 1.2576x; 1.2576x over previous
"""GCN message-passing kernel (nn_Encoder_953482739902) for 8 TRN2 NeuronCores.

Computation (PyG GCNConv + mish):
    deg  = in-degree(col) + 1 (self-loops)
    dinv = deg^-1/2
    agg[t] = sum_{e: col(e)=t} dinv[row_e] * x[row_e] + dinv[t] * x[t]
    out  = mish(dinv[t] * (agg @ W) + b)

Distribution: targets (and output rows) sharded 8 ways; each core owns 12500
targets and the edges pointing at them (edge partition by target node). Every
core receives the full packed feature table.

Per-core device pipeline:
  - sources gathered via dma_gather (int16 indices; the 100k-row table is
    split in 4 chunks of 32768 rows to fit int16 indexing). deg(src) rides
    along mantissa-packed into the low 8 bits of feature 0, extracted on
    device (int ops + Ln/Exp for deg^-1/2).
  - segment-sum over targets via valued one-hot matmuls (O[e, t] =
    (iota==tloc)*dinv_src) accumulating in a PSUM bank per 512-target window.
  - finalize: agg @ W, scale by dinv_tgt, exact mish via
    z * a / (a + 2) with a = u^2 + 2u, u = exp(z).

Host side does index work only: bincount (degree), sorting/partitioning,
int casts, layout packing. All floating-point math runs on device.
"""

import numpy as np

N_NODES = 100000
IN_CH = 128
N_CORES = 8
TPC = 12500              # targets per core
TPAD = 12800             # padded (25 windows x 512)
NW = TPAD // 512         # 25 PSUM windows per core
NCHUNK = 4
CHUNK = 25000
NROWS_PAD = N_NODES
GU_MAX = 16              # subtiles (128 idx each) per dma_gather


def _build_schedule(row, col):
    """Partition/sort edges; return per-core host arrays + static sizes.

    row/col: int64 [E_tot] including self-loops.
    Static shapes (shared across cores): per (window, chunk) slot subtile
    count = max over cores, rounded up.
    """
    E_tot = row.shape[0]
    core = col // TPC
    tgt_local = (col - core * TPC).astype(np.int64)
    window = tgt_local >> 9
    chunk = row // CHUNK

    # counts per (core, window, chunk)
    key = ((core * NW + window) * NCHUNK + chunk).astype(np.int64)
    counts = np.bincount(key, minlength=N_CORES * NW * NCHUNK).reshape(
        N_CORES, NW, NCHUNK
    )
    s_slot = -(-counts.max(axis=0) // 128)  # [NW, NCHUNK] subtiles

    # per-slot gather units of <= GU_MAX subtiles
    gu_list = []  # (w, c, subtile_base_global, n_subtiles)
    sub_base = np.zeros((NW, NCHUNK), np.int64)
    nsub_tot = 0
    for w in range(NW):
        for c in range(NCHUNK):
            sub_base[w, c] = nsub_tot
            nsub_tot += int(s_slot[w, c])
    for w in range(NW):
        for c in range(NCHUNK):
            s = int(s_slot[w, c])
            off = 0
            while off < s:
                su = min(GU_MAX, s - off)
                gu_list.append((w, c, int(sub_base[w, c]) + off, su))
                off += su

    # order of edges inside the concatenated per-core stream
    order = np.lexsort((tgt_local, chunk, window, core))
    ro, tl, wo, co, ko = (
        row[order], tgt_local[order], window[order], core[order], chunk[order]
    )

    idx_cols = nsub_tot * 8  # int16 cols: 128 idx/subtile / 16
    idx16 = np.zeros((N_CORES, 128, idx_cols), np.int16)
    tloc = np.full((N_CORES, 128, nsub_tot), -1.0, np.float32)

    # slot start offsets within the sorted stream, per core
    cum = np.zeros((N_CORES, NW, NCHUNK + 1), np.int64)
    cum[:, :, 1:] = np.cumsum(counts, axis=2)
    wcum = np.zeros((N_CORES, NW + 1), np.int64)
    wcum[:, 1:] = np.cumsum(counts.sum(axis=2), axis=1)
    ccum = np.zeros(N_CORES + 1, np.int64)
    ccum[1:] = np.cumsum(counts.sum(axis=(1, 2)))

    for cc in range(N_CORES):
        for w in range(NW):
            for ch in range(NCHUNK):
                a = ccum[cc] + wcum[cc, w] + cum[cc, w, ch]
                b = ccum[cc] + wcum[cc, w] + cum[cc, w, ch + 1]
                n = b - a
                if n == 0:
                    continue
                srcs = (ro[a:b] - ch * CHUNK).astype(np.int16)
                assert n <= int(s_slot[w, ch]) * 128
                tls = (tl[a:b] - (w << 9)).astype(np.float32)
                sb = int(sub_base[w, ch])
                npad = int(s_slot[w, ch]) * 128
                sv = np.zeros(npad, np.int16)
                tv = np.full(npad, -1.0, np.float32)
                sv[:n] = srcs
                tv[:n] = tls
                # idx: logical i -> (i%16, i//16), replicated 8x over partitions
                wrapped = sv.reshape(-1, 16).T  # [16, npad/16]
                idx16[cc, :, sb * 8 : sb * 8 + npad // 16] = np.tile(wrapped, (8, 1))
                # tloc: logical i -> (partition i%128, subtile i//128)
                tloc[cc, :, sb : sb + npad // 128] = tv.reshape(-1, 128).T

    return idx16, tloc, gu_list, nsub_tot, int(s_slot.sum())


def _fix_act_table_loads(nc):
    """All activations used here (Ln, Exp, Copy-family) live in the single
    'natural_log_exp_and_others' table set; the default per-function chooser
    alternates between sets, inserting a ~100us table reload before nearly
    every activation. Retarget every load to the combined set and drop the
    now-redundant repeats (keep the first load per basic block)."""
    import concourse.mybir as mybir
    from concourse.hw_specs import get_activation_tables

    tables = get_activation_tables(nc.m.arch)
    names = list(tables.keys())
    target = "natural_log_exp_and_others"
    target_id = names.index(target)
    allowed = tables[target]
    for f in nc.m.functions:
        for blk in f.blocks:
            insts = blk.instructions
            for inst in insts:
                if isinstance(inst, mybir.InstActivation):
                    assert inst.func in allowed, inst.func
            kept = []
            seen_load = False
            for inst in insts:
                if isinstance(inst, mybir.InstLoadActFuncSet):
                    si = inst.sync_info
                    has_sync = si is not None and (si.on_wait or si.on_update)
                    if seen_load and not has_sync:
                        continue
                    inst.act_func_set_id = target_id
                    seen_load = True
                kept.append(inst)
            if len(kept) != len(insts):
                insts[:] = kept


def _build_bass(nsub_tot, gu_list, has_bias):
    import concourse.bacc as bacc
    import concourse.mybir as mybir
    from concourse.tile import TileContext

    AF = mybir.ActivationFunctionType
    OP = mybir.AluOpType

    nc = bacc.Bacc("TRN2", target_bir_lowering=False, debug=False,
                   num_devices=N_CORES)
    _orig_compile = nc.compile

    def _compile_with_fix():
        _orig_compile()
        _fix_act_table_loads(nc)

    nc.compile = _compile_with_fix
    xpk_d = nc.dram_tensor("xpk", [NROWS_PAD, IN_CH], mybir.dt.float32,
                           kind="ExternalInput")
    idx_d = nc.dram_tensor("idx16", [128, nsub_tot * 8], mybir.dt.int16,
                           kind="ExternalInput")
    tloc_d = nc.dram_tensor("tloc", [128, nsub_tot], mybir.dt.float32,
                            kind="ExternalInput")
    iota_d = nc.dram_tensor("iota512", [128, 512], mybir.dt.float32,
                            kind="ExternalInput")
    w_d = nc.dram_tensor("Wm", [IN_CH, IN_CH], mybir.dt.float32,
                         kind="ExternalInput")
    degt_d = nc.dram_tensor("degt", [128, TPAD // 128], mybir.dt.float32,
                            kind="ExternalInput")
    if has_bias:
        b_d = nc.dram_tensor("bb", [128, 512], mybir.dt.float32,
                             kind="ExternalInput")
    out_d = nc.dram_tensor("out", [TPAD, IN_CH], mybir.dt.float32,
                           kind="ExternalOutput")

    with TileContext(nc) as tc:
        with (
            tc.tile_pool(name="const", bufs=1) as cp,
            tc.tile_pool(name="gbuf", bufs=5) as gp,
            tc.tile_pool(name="obuf", bufs=8) as op_,
            tc.tile_pool(name="small", bufs=5) as sp,
            tc.tile_pool(name="fin", bufs=3) as fp,
            tc.tile_pool(name="psw", bufs=2, space="PSUM") as pwp,
            tc.tile_pool(name="ps2", bufs=2, space="PSUM") as p2p,
        ):
            iota = cp.tile([128, 512], mybir.dt.float32)
            nc.sync.dma_start(out=iota[:], in_=iota_d[:])
            wsb = cp.tile([IN_CH, IN_CH], mybir.dt.float32)
            nc.sync.dma_start(out=wsb[:], in_=w_d[:])
            idxt = cp.tile([128, nsub_tot * 8], mybir.dt.int16)
            nc.sync.dma_start(out=idxt[:], in_=idx_d[:])
            tlct = cp.tile([128, nsub_tot], mybir.dt.float32)
            nc.sync.dma_start(out=tlct[:], in_=tloc_d[:])
            degt = cp.tile([128, TPAD // 128], mybir.dt.float32)
            nc.sync.dma_start(out=degt[:], in_=degt_d[:])
            if has_bias:
                bsb = cp.tile([128, 512], mybir.dt.float32)
                nc.sync.dma_start(out=bsb[:], in_=b_d[:])
            lnt = cp.tile([128, TPAD // 128], mybir.dt.float32)
            nc.scalar.activation(lnt[:], degt[:], AF.Ln)
            dinvt = cp.tile([128, TPAD // 128], mybir.dt.float32)
            nc.scalar.activation(dinvt[:], lnt[:], AF.Exp, scale=-0.5)

            # group gather units by window
            gus_by_w = [[] for _ in range(NW)]
            for (w, c, sb, su) in gu_list:
                gus_by_w[w].append((c, sb, su))

            for w in range(NW):
                psw = pwp.tile([128, 512], mybir.dt.float32, tag="psw",
                               space="PSUM")
                n_mm = sum(su for (_, _, su) in gus_by_w[w])
                mm = 0
                for (c, sb, su) in gus_by_w[w]:
                    g = gp.tile([128, GU_MAX, 128], mybir.dt.float32, tag="g")
                    nc.gpsimd.dma_gather(
                        out_ap=g[:, :su, :],
                        in_ap=xpk_d[c * CHUNK : (c + 1) * CHUNK, :],
                        idxs_ap=idxt[:, sb * 8 : (sb + su) * 8],
                        num_idxs=su * 128,
                        num_idxs_reg=su * 128,
                        elem_size=IN_CH,
                        single_packet=False,
                    )
                    degs_i = sp.tile([128, GU_MAX], mybir.dt.int32, tag="di")
                    nc.vector.tensor_scalar(
                        out=degs_i[:, :su],
                        in0=g[:, :su, 0].bitcast(mybir.dt.int32),
                        scalar1=0xFF, scalar2=None, op0=OP.bitwise_and)
                    degs_f = sp.tile([128, GU_MAX], mybir.dt.float32, tag="df")
                    nc.vector.tensor_copy(out=degs_f[:, :su], in_=degs_i[:, :su])
                    lns = sp.tile([128, GU_MAX], mybir.dt.float32, tag="ln")
                    nc.scalar.activation(lns[:, :su], degs_f[:, :su], AF.Ln)
                    dinvs = sp.tile([128, GU_MAX], mybir.dt.float32, tag="dv")
                    nc.scalar.activation(dinvs[:, :su], lns[:, :su], AF.Exp,
                                         scale=-0.5)
                    for j in range(su):
                        O = op_.tile([128, 512], mybir.dt.float32, tag="O")
                        ts_engine = nc.any if (j % 2 == 0) else nc.vector
                        ts_engine.tensor_scalar(
                            out=O[:], in0=iota[:],
                            scalar1=tlct[:, sb + j : sb + j + 1],
                            scalar2=dinvs[:, j : j + 1],
                            op0=OP.is_equal, op1=OP.mult)
                        nc.tensor.matmul(out=psw[:], lhsT=g[:, j, :], rhs=O[:],
                                         start=(mm == 0), stop=(mm == n_mm - 1))
                        mm += 1

                accs = fp.tile([128, 512], mybir.dt.float32, tag="acc")
                nc.vector.tensor_copy(out=accs[:], in_=psw[:])
                ps2 = p2p.tile([128, 512], mybir.dt.float32, tag="ps2",
                               space="PSUM")
                zt = fp.tile([128, 512], mybir.dt.float32, tag="zt")
                for j4 in range(4):
                    sl = slice(j4 * 128, (j4 + 1) * 128)
                    nc.tensor.matmul(out=ps2[:, sl], lhsT=accs[:, sl],
                                     rhs=wsb[:], start=True, stop=True)
                    nc.vector.tensor_scalar(
                        out=zt[:, sl], in0=ps2[:, sl],
                        scalar1=dinvt[:, 4 * w + j4 : 4 * w + j4 + 1],
                        scalar2=None, op0=OP.mult)
                if has_bias:
                    nc.vector.tensor_tensor(out=zt[:], in0=zt[:], in1=bsb[:],
                                            op=OP.add)
                u = fp.tile([128, 512], mybir.dt.float32, tag="u")
                nc.scalar.activation(u[:], zt[:], AF.Exp)
                a1 = fp.tile([128, 512], mybir.dt.float32, tag="a1")
                nc.vector.tensor_scalar(out=a1[:], in0=u[:], scalar1=2.0,
                                        scalar2=None, op0=OP.add)
                a = fp.tile([128, 512], mybir.dt.float32, tag="a")
                nc.vector.tensor_tensor(out=a[:], in0=a1[:], in1=u[:],
                                        op=OP.mult)
                den = fp.tile([128, 512], mybir.dt.float32, tag="den")
                nc.vector.tensor_scalar(out=den[:], in0=a[:], scalar1=2.0,
                                        scalar2=None, op0=OP.add)
                rden = fp.tile([128, 512], mybir.dt.float32, tag="rden")
                nc.vector.reciprocal_approx_fast(out=rden[:], in_=den[:])
                m = fp.tile([128, 512], mybir.dt.float32, tag="m")
                nc.vector.tensor_tensor(out=m[:], in0=a[:], in1=rden[:],
                                        op=OP.mult)
                mz = fp.tile([128, 512], mybir.dt.float32, tag="mz")
                nc.vector.tensor_tensor(out=mz[:], in0=m[:], in1=zt[:],
                                        op=OP.mult)
                for j4 in range(4):
                    nc.sync.dma_start(
                        out=out_d[w * 512 + j4 * 128 : w * 512 + (j4 + 1) * 128, :],
                        in_=mz[:, j4 * 128 : (j4 + 1) * 128])
    nc.finalize()
    return nc


class _Runner:
    """PJRT runner (axon): jit once, device-resident inputs, reusable."""

    def __init__(self, nc):
        import jax
        import concourse.mybir as mybir
        from jax.sharding import Mesh, PartitionSpec
        from jax.experimental.shard_map import shard_map
        from concourse import bass2jax
        from concourse.bass2jax import _bass_exec_p, install_neuronx_cc_hook

        install_neuronx_cc_hook()
        self.nc = nc
        partition_name = (
            nc.partition_id_tensor.name if nc.partition_id_tensor else None
        )
        in_names, out_names, out_avals, zero_outs = [], [], [], []
        for alloc in nc.m.functions[0].allocations:
            if not isinstance(alloc, mybir.MemoryLocationSet):
                continue
            name = alloc.memorylocations[0].name
            if alloc.kind == "ExternalInput":
                if name != partition_name:
                    in_names.append(name)
            elif alloc.kind == "ExternalOutput":
                shape = tuple(alloc.tensor_shape)
                dtype = mybir.dt.np(alloc.dtype)
                out_names.append(name)
                out_avals.append(jax.core.ShapedArray(shape, dtype))
                zero_outs.append(np.zeros(shape, dtype))
        self.in_names, self.out_names = in_names, out_names
        all_in = list(in_names) + list(out_names)
        if partition_name is not None:
            all_in.append(partition_name)

        def _body(*args):
            operands = list(args)
            if partition_name is not None:
                operands.append(bass2jax.partition_id_tensor())
            return tuple(_bass_exec_p.bind(
                *operands,
                out_avals=tuple(out_avals),
                in_names=tuple(all_in),
                out_names=tuple(out_names),
                lowering_input_output_aliases=(),
                sim_require_finite=True,
                sim_require_nnan=True,
                nc=nc,
            ))

        devices = jax.devices()[:N_CORES]
        mesh = Mesh(np.asarray(devices), ("core",))
        n_in = len(in_names) + len(out_names)
        self.fn = jax.jit(
            shard_map(_body, mesh=mesh,
                      in_specs=(PartitionSpec("core"),) * n_in,
                      out_specs=(PartitionSpec("core"),) * len(out_names),
                      check_rep=False),
            keep_unused=True)
        self.zero_outs = zero_outs
        self.jax = jax

    def stage(self, in_maps):
        args = []
        for name in self.in_names:
            args.append(np.concatenate(
                [np.asarray(m[name]) for m in in_maps], axis=0))
        for z in self.zero_outs:
            args.append(np.concatenate([z] * N_CORES, axis=0))
        self._dev_args = [self.jax.device_put(a) for a in args]
        for a in self._dev_args:
            a.block_until_ready()

    def run(self):
        outs = self.fn(*self._dev_args)
        for o in outs:
            o.block_until_ready()
        return outs

    def results(self, outs):
        per_core = [dict() for _ in range(N_CORES)]
        for i, name in enumerate(self.out_names):
            arr = np.asarray(outs[i])
            for c, piece in enumerate(np.split(arr, N_CORES, axis=0)):
                per_core[c][name] = piece
        return per_core


_CACHE = {}


def _prepare(x, edge_index, W, b):
    x = np.asarray(x, dtype=np.float32)
    edge_index = np.asarray(edge_index)
    W = np.asarray(W, dtype=np.float32)
    b = np.asarray(b, dtype=np.float32)
    N = x.shape[0]
    assert N == N_NODES and x.shape[1] == IN_CH

    row = edge_index[0].astype(np.int64)
    col = edge_index[1].astype(np.int64)
    loops = np.arange(N, dtype=np.int64)
    row_all = np.concatenate([row, loops])
    col_all = np.concatenate([col, loops])

    deg = np.bincount(col_all, minlength=N).astype(np.int64)
    assert deg.max() <= 255, "degree exceeds 8-bit mantissa packing"

    # packed feature table: deg in low 8 mantissa bits of feature 0
    xpk = np.zeros((NROWS_PAD, IN_CH), np.float32)
    xpk[:N] = x
    bits = xpk[:, 0].view(np.int32)
    bits[:N] = (bits[:N] & ~np.int32(0xFF)) | deg.astype(np.int32)
    bits[N:] = 1  # harmless deg=1 for padding rows

    idx16, tloc, gu_list, nsub_tot, _ = _build_schedule(row_all, col_all)

    degt = np.ones((N_CORES, 128, TPAD // 128), np.float32)
    for c in range(N_CORES):
        d = deg[c * TPC : (c + 1) * TPC].astype(np.float32)
        dp = np.ones(TPAD, np.float32)
        dp[:TPC] = d
        degt[c] = dp.reshape(-1, 128).T

    iota512 = np.broadcast_to(
        np.arange(512, dtype=np.float32), (128, 512)).copy()
    has_bias = bool(np.any(b != 0))

    in_maps = []
    for c in range(N_CORES):
        m = {
            "xpk": xpk,
            "idx16": np.ascontiguousarray(idx16[c]),
            "tloc": np.ascontiguousarray(tloc[c]),
            "iota512": iota512,
            "Wm": W,
            "degt": np.ascontiguousarray(degt[c]),
        }
        if has_bias:
            m["bb"] = np.tile(b[None, :], (128, 4)).astype(np.float32)
        in_maps.append(m)

    key = (nsub_tot, tuple(gu_list), has_bias)
    if key not in _CACHE:
        nc = _build_bass(nsub_tot, gu_list, has_bias)
        runner = _Runner(nc)
        _CACHE.clear()
        _CACHE[key] = runner
    return _CACHE[key], in_maps


def kernel(x, edge_index, W, b):
    runner, in_maps = _prepare(x, edge_index, W, b)
    runner.stage(in_maps)
    outs = runner.run()
    res = runner.results(outs)
    return np.concatenate(
        [res[c]["out"][:TPC] for c in range(N_CORES)], axis=0)
